# revision 27
# baseline (speedup 1.0000x reference)
"""Multi-head attention (B=4, L=2048, d_model=1024, 16 heads) on 8 TRN2 NeuronCores.

Sharding: core c handles batch b = c//2 and head-group g = c%2 (8 heads each).
Column-parallel QKV projections, per-head attention, row-parallel out-projection;
the host sums the two partial outputs per batch and adds the output bias.

v3 structure:
  - Scores per head via K=64 matmuls at tile_position (0,0)/(64,0): adjacent
    A/B-head matmuls occupy disjoint row-groups and run concurrently, so the
    score phase streams fully dense (no structural zeros) -- 2x fewer PE
    cycles than block-diag packing.
  - Softmax denominators: VectorE accumulates S[:,h,:] = sum_kc exp-tiles in
    bf16 (2x DVE mode; per-partition rounding errors average out in the
    128-partition reduction), then one [128,64]-of-ones matmul per head
    reduces partitions AND broadcasts the rowsum; reciprocal_approx_fast +
    one multiply normalize.  No per-chunk `ones` matmuls, no DRAM bounce.
  - exp batched as [128,1536] ScalarE activations over 3-bank PSUM score
    tiles (amortizes the ~352-cycle ACT instruction overhead).
  - ctx pair accumulates in ONE psum bank: per-head accumulation groups on
    disjoint partition ranges (the start=True pending-zero marking is
    per-partition-range x bank).
  - x pipeline: per-x cast DMA (gpsimd software DGE, f32->bf16 DRAM bounce;
    3 in flight -- per-queue FIFO drains them in issue order) then xbar
    transposes ALL on the sync queue: two DMA_TRANSPOSEs in flight on
    different hwdge queues corrupt each other (shared xbar state).
  - load/compute order xk -> xq -> xv, projections K -> Q -> V: attention
    starts once K + the first Q chunk are projected and consumes V chunks
    as the V projection emits them.
  - out-projection in 2-wide psum groups with batched [128,1024] copies
    (DVE/ACT alternating) and one 512KB DMA per group.
"""

import ml_dtypes
import numpy as np

BF16_NP = np.dtype(ml_dtypes.bfloat16)

import concourse.bass as bass
import concourse.tile as tile
from concourse import mybir, bacc
from concourse.bass_utils import run_bass_kernel_spmd

F32 = mybir.dt.float32
BF16 = mybir.dt.bfloat16

L = 2048          # sequence length
D = 1024          # d_model
CC = 512          # columns per core (8 heads x 64)
DK = 64           # head dim
P = 128           # partitions
NP = 4            # head pairs per core
SCALE = 1.0 / np.sqrt(DK)
NKC = L // P      # 16 key chunks of 128 keys
NSUB = 2 * NKC    # 32 (head, kc) subs per (pair, q-chunk)
NC_ = 4           # 512-token chunks


def build_attention_core(nc, tc, pools):
    sb1, xtp, ptp, osp, misc, dram, big, ctxp = pools

    xq = nc.dram_tensor("xq", [L, D], BF16, kind="ExternalInput").ap()
    xk = nc.dram_tensor("xk", [L, D], BF16, kind="ExternalInput").ap()
    xv = nc.dram_tensor("xv", [L, D], BF16, kind="ExternalInput").ap()
    wq = nc.dram_tensor("wq", [D, CC], BF16, kind="ExternalInput").ap()
    wk = nc.dram_tensor("wk", [D, CC], BF16, kind="ExternalInput").ap()
    wv = nc.dram_tensor("wv", [D, CC], BF16, kind="ExternalInput").ap()
    wo = nc.dram_tensor("wo", [CC, D], BF16, kind="ExternalInput").ap()
    bq = nc.dram_tensor("bq", [CC], F32, kind="ExternalInput").ap()
    bk = nc.dram_tensor("bk", [CC], F32, kind="ExternalInput").ap()
    bv = nc.dram_tensor("bv", [CC], F32, kind="ExternalInput").ap()
    out = nc.dram_tensor("out", [D, L], BF16, kind="ExternalOutput").ap()

    EXP = mybir.ActivationFunctionType.Exp
    MULT = mybir.AluOpType.mult
    ADD = mybir.AluOpType.add

    wq_sb = sb1.tile([P, D // P, CC], BF16, tag="wq")
    wk_sb = sb1.tile([P, D // P, CC], BF16, tag="wk")
    wv_sb = sb1.tile([P, D // P, CC], BF16, tag="wv")
    wo_sb = sb1.tile([P, CC // P, D], BF16, tag="wo")

    bq_sb = sb1.tile([P, NP], F32, tag="bq")
    bk_sb = sb1.tile([P, NP], F32, tag="bk")
    nc.sync.dma_start(bq_sb[:], bq.rearrange("(o p) -> p o", p=P))
    nc.sync.dma_start(bk_sb[:], bk.rearrange("(o p) -> p o", p=P))
    bv_row = sb1.tile([1, CC], BF16, tag="bv")
    bv_f32 = sb1.tile([1, CC], F32, tag="bvf")
    nc.scalar.dma_start(bv_f32[:], bv[None, :])
    nc.vector.tensor_copy(bv_row[:], bv_f32[:])

    ones_row = sb1.tile([1, P], BF16, tag="ones_row")   # K=1 lhsT for V bias
    nc.vector.memset(ones_row[:], 1.0)
    ones64 = sb1.tile([P, DK], BF16, tag="ones64")      # rowsum+broadcast lhsT
    nc.vector.memset(ones64[:], 1.0)

    def load_x(x):
        """Transpose host-pre-cast bf16 x straight from DRAM, sync queue
        only: two DMA_TRANSPOSEs in flight on different hwdge queues corrupt
        each other (shared xbar state)."""
        xts = [xtp.tile([P, L], BF16, tag="xt", name=f"xt{i}")
               for i in range(D // P)]
        for c in range(D // P):
            nc.sync.dma_start_transpose(xts[c][:], x[:, c * P:(c + 1) * P])
        return xts

    qt_sb = sb1.tile([P, NP, L], BF16, tag="qt")   # [col-in-pair, pair, tok]
    kt_sb = sb1.tile([P, NP, L], BF16, tag="kt")   # [dk(A:0:64,B:64:128), pair, key]
    v_sb = sb1.tile([P, NKC, 8, DK], BF16, tag="v")  # [key, kc, head, dim]
    ct_sb = sb1.tile([P, NP, L], BF16, tag="ct")   # [col-in-pair, pair, tok]

    def load_w(w_sb, w, eng):
        eng.dma_start(w_sb[:], w.rearrange("(o p) c -> p o c", p=P))

    def proj_batch(w_sb, xts, dst, bias, p, tns):
        """One [128, len(tns)*512] psum group per (pair, tn-group): fewer
        big-pool allocations gating attention's score tiles, one batched
        bias-add."""
        n = len(tns)
        acc = big.tile([P, n, 512], F32, tag="sc", name="pacc")
        for j, tn in enumerate(tns):
            for c in range(D // P):
                nc.tensor.matmul(acc[:, j, :], w_sb[:, c, p * P:(p + 1) * P],
                                 xts[c][:, tn * 512:(tn + 1) * 512],
                                 start=(c == 0), stop=(c == D // P - 1),
                                 skip_group_check=(j > 0))
        t0 = tns[0]
        nc.vector.tensor_scalar_add(dst[:, p, t0 * 512:(t0 + n) * 512],
                                    acc[:], bias[:, p:p + 1])

    # K projection (p-major so pair 0 completes first)
    xts_k = load_x(xk)
    load_w(wk_sb, wk, nc.scalar)
    for p in range(NP):
        proj_batch(wk_sb, xts_k, kt_sb, bk_sb, p, (0, 1, 2))
        proj_batch(wk_sb, xts_k, kt_sb, bk_sb, p, (3,))

    # Q projection: transpose+project only the first 512 tokens now (all
    # attention qh0 needs), defer the rest until after xv's transposes so
    # the V projection isn't stuck behind them on the sync queue.
    xts_q = [xtp.tile([P, L], BF16, tag="xt", name=f"xtq{i}")
             for i in range(D // P)]
    for c in range(D // P):
        nc.sync.dma_start_transpose(xts_q[c][:, 0:512],
                                    xq[0:512, c * P:(c + 1) * P])
    load_w(wq_sb, wq, nc.scalar)
    for p in range(NP):
        proj_batch(wq_sb, xts_q, qt_sb, bq_sb, p, (0,))

    # V projection (tn-major; psum from the ctx pool so it doesn't contend
    # with attention's score tiles), straight into v_sb; attention consumes
    # v_sb[:, kc] chunks as they land.
    xts_v = load_x(xv)
    load_w(wv_sb, wv, nc.scalar)
    load_w(wo_sb, wo, nc.scalar)
    for tn in range(L // P):
        acc = ctxp.tile([P, 512], F32, tag="ctx", name="vacc")
        for c in range(D // P):
            nc.tensor.matmul(acc[:], xts_v[c][:, tn * P:(tn + 1) * P],
                             wv_sb[:, c, :], start=(c == 0), stop=False)
        nc.tensor.matmul(acc[:], ones_row[:], bv_row[:], start=False, stop=True)
        av = acc.rearrange("p (h m) -> p h m", m=DK)   # [128, 8, 64]
        nc.vector.tensor_copy(v_sb[:, tn, :, :], av)

    # rest of Q's transposes + projections (needed from qh1 onward)
    for c in range(D // P):
        nc.sync.dma_start_transpose(xts_q[c][:, 512:L],
                                    xq[512:L, c * P:(c + 1) * P])
    for p in range(NP):
        proj_batch(wq_sb, xts_q, qt_sb, bq_sb, p, (1, 2, 3))

    # ---- attention ----
    for qh in range(4):
        qs = slice(qh * 512, (qh + 1) * 512)
        for p in range(NP):
            ctx = ctxp.tile([P, 512], F32, tag="ctx")
            # S[:, h, :] accumulates sum_kc exp-tiles for head h in bf16.
            S = misc.tile([P, 2, 512], BF16, tag="S", bufs=4)
            s0 = 0
            while s0 < NSUB:
                ns = min(3, NSUB - s0)
                sc = big.tile([P, ns, 512], F32, tag="sc")
                for i in range(ns):
                    h, kc = (s0 + i) % 2, (s0 + i) // 2
                    hs = slice(DK * h, DK * (h + 1))
                    nc.tensor.matmul(sc[:, i, :],
                                     kt_sb[hs, p, kc * P:(kc + 1) * P],
                                     qt_sb[hs, p, qs], start=True, stop=True)
                pt = ptp.tile([P, ns, 512], BF16, tag="pt")
                nc.scalar.activation(pt[:], sc[:], EXP, scale=SCALE)
                for i in range(ns):
                    s = s0 + i
                    h, kc = s % 2, s // 2
                    nc.tensor.matmul(ctx[DK * h:DK * (h + 1), :],
                                     v_sb[:, kc, 2 * p + h, :], pt[:, i, :],
                                     start=(kc == 0),
                                     stop=(kc == NKC - 1),
                                     skip_group_check=(h == 1))
                # S accumulation: one paired (A,B) add + one single add per
                # tile; the pair starts at the even-s slot.
                j = 0 if s0 % 2 == 0 else 1
                if s0 == 0:
                    nc.vector.tensor_copy(S[:, 0:2, :], pt[:, 0:2, :])
                else:
                    nc.vector.tensor_tensor(S[:, 0:2, :], S[:, 0:2, :],
                                            pt[:, j:j + 2, :], ADD)
                if ns == 3:
                    k = 2 if j == 0 else 0
                    hk = (s0 + k) % 2
                    nc.vector.tensor_tensor(S[:, hk, :], S[:, hk, :],
                                            pt[:, k, :], ADD)
                s0 += ns
            # rowsum (partition reduce) + broadcast via all-ones matmul
            rb = ctxp.tile([P, 512], F32, tag="ctx")
            nc.tensor.matmul(rb[0:DK, :], ones64[:], S[:, 0, :],
                             start=True, stop=True)
            nc.tensor.matmul(rb[DK:P, :], ones64[:], S[:, 1, :],
                             start=True, stop=True, skip_group_check=True)
            rec = misc.tile([P, 512], F32, tag="rec", bufs=2)
            nc.vector.reciprocal_approx_fast(rec[:], rb[:])
            nc.vector.tensor_tensor(ct_sb[:, p, qs], ctx[:], rec[:], MULT)

    # ---- out projection: outT[oc, tok], 2-wide psum groups ----
    for g in range(16):
        oc, tn0 = g // 2, (g % 2) * 2
        acc = big.tile([P, 2, 512], F32, tag="sc")
        for j in range(2):
            tn = tn0 + j
            for p in range(NP):
                nc.tensor.matmul(acc[:, j, :],
                                 wo_sb[:, p, oc * P:(oc + 1) * P],
                                 ct_sb[:, p, tn * 512:(tn + 1) * 512],
                                 start=(p == 0), stop=(p == NP - 1))
        o_sb = osp.tile([P, 2, 512], BF16, tag="o")
        if g % 2 == 0:
            nc.vector.tensor_copy(o_sb[:], acc[:])
        else:
            nc.scalar.copy(o_sb[:], acc[:])
        nc.sync.dma_start(out[oc * P:(oc + 1) * P, tn0 * 512:(tn0 + 2) * 512],
                          o_sb[:])


def build_bass():
    nc = bacc.Bacc("TRN2", num_devices=8, debug=False)
    with tile.TileContext(nc) as tc:
        with (
            tc.tile_pool(name="sb1", bufs=1) as sb1,
            tc.tile_pool(name="xtp", bufs=16) as xtp,
            tc.tile_pool(name="ptp", bufs=6) as ptp,
            tc.tile_pool(name="osp", bufs=4) as osp,
            tc.tile_pool(name="misc", bufs=1) as misc,
            tc.tile_pool(name="dram", bufs=1, space="DRAM") as dram,
            tc.tile_pool(name="big", bufs=2, space="PSUM") as big,
            tc.tile_pool(name="ctxp", bufs=2, space="PSUM") as ctxp,
        ):
            build_attention_core(
                nc, tc, (sb1, xtp, ptp, osp, misc, dram, big, ctxp))
    nc.compile()
    return nc


_CACHE = {}


def _get_nc():
    if "nc" not in _CACHE:
        _CACHE["nc"] = build_bass()
    return _CACHE["nc"]


def make_in_maps(query, key, value, Wq, bq, Wk, bk, Wv, bv, Wo):
    f = np.ascontiguousarray
    in_maps = []
    for c in range(8):
        b, g = c // 2, c % 2
        cs = slice(g * CC, (g + 1) * CC)
        in_maps.append({
            "xq": f(query[b], dtype=BF16_NP),
            "xk": f(key[b], dtype=BF16_NP),
            "xv": f(value[b], dtype=BF16_NP),
            "wq": f(Wq[:, cs], dtype=BF16_NP),
            "wk": f(Wk[:, cs], dtype=BF16_NP),
            "wv": f(Wv[:, cs], dtype=BF16_NP),
            "wo": f(Wo[cs, :], dtype=BF16_NP),
            "bq": f(bq[cs], dtype=np.float32),
            "bk": f(bk[cs], dtype=np.float32),
            "bv": f(bv[cs], dtype=np.float32),
        })
    return in_maps


def kernel(query, key, value, Wq, bq, Wk, bk, Wv, bv, Wo, bo, **run_kwargs):
    query, key, value = np.asarray(query), np.asarray(key), np.asarray(value)
    Wq, Wk, Wv, Wo = np.asarray(Wq), np.asarray(Wk), np.asarray(Wv), np.asarray(Wo)
    bq, bk, bv, bo = np.asarray(bq), np.asarray(bk), np.asarray(bv), np.asarray(bo)
    nc = _get_nc()
    in_maps = make_in_maps(query, key, value, Wq, bq, Wk, bk, Wv, bv, Wo)
    res = run_bass_kernel_spmd(nc, in_maps, core_ids=list(range(8)), **run_kwargs)
    B = query.shape[0]
    out = np.empty((B, L, D), np.float32)
    for b in range(B):
        acc = (res.results[2 * b]["out"].astype(np.float32).T
               + res.results[2 * b + 1]["out"].astype(np.float32).T)
        out[b] = acc + bo[None, :].astype(np.float32)
    if run_kwargs:
        kernel.last_results = res
    return out


# revision 28
# speedup vs baseline: 1.0218x; 1.0218x over previous
"""Multi-head attention (B=4, L=2048, d_model=1024, 16 heads) on 8 TRN2 NeuronCores.

Sharding: core c handles batch b = c//2 and head-group g = c%2 (8 heads each).
Column-parallel QKV projections, per-head attention, row-parallel out-projection;
the host sums the two partial outputs per batch and adds the output bias.

v3 structure:
  - Scores per head via K=64 matmuls at tile_position (0,0)/(64,0): adjacent
    A/B-head matmuls occupy disjoint row-groups and run concurrently, so the
    score phase streams fully dense (no structural zeros) -- 2x fewer PE
    cycles than block-diag packing.
  - Softmax denominators: VectorE accumulates S[:,h,:] = sum_kc exp-tiles in
    bf16 (2x DVE mode; per-partition rounding errors average out in the
    128-partition reduction), then one [128,64]-of-ones matmul per head
    reduces partitions AND broadcasts the rowsum; reciprocal_approx_fast +
    one multiply normalize.  No per-chunk `ones` matmuls, no DRAM bounce.
  - exp batched as [128,1536] ScalarE activations over 3-bank PSUM score
    tiles (amortizes the ~352-cycle ACT instruction overhead).
  - ctx pair accumulates in ONE psum bank: per-head accumulation groups on
    disjoint partition ranges (the start=True pending-zero marking is
    per-partition-range x bank).
  - x pipeline: per-x cast DMA (gpsimd software DGE, f32->bf16 DRAM bounce;
    3 in flight -- per-queue FIFO drains them in issue order) then xbar
    transposes ALL on the sync queue: two DMA_TRANSPOSEs in flight on
    different hwdge queues corrupt each other (shared xbar state).
  - load/compute order xk -> xq -> xv, projections K -> Q -> V: attention
    starts once K + the first Q chunk are projected and consumes V chunks
    as the V projection emits them.
  - out-projection in 2-wide psum groups with batched [128,1024] copies
    (DVE/ACT alternating) and one 512KB DMA per group.
"""

import ml_dtypes
import numpy as np

BF16_NP = np.dtype(ml_dtypes.bfloat16)

import concourse.bass as bass
import concourse.tile as tile
from concourse import mybir, bacc
from concourse.bass_utils import run_bass_kernel_spmd

F32 = mybir.dt.float32
BF16 = mybir.dt.bfloat16

L = 2048          # sequence length
D = 1024          # d_model
CC = 512          # columns per core (8 heads x 64)
DK = 64           # head dim
P = 128           # partitions
NP = 4            # head pairs per core
SCALE = 1.0 / np.sqrt(DK)
NKC = L // P      # 16 key chunks of 128 keys
NSUB = 2 * NKC    # 32 (head, kc) subs per (pair, q-chunk)
NC_ = 4           # 512-token chunks


def build_attention_core(nc, tc, pools):
    sb1, xtp, ptp, osp, misc, dram, big, ctxp = pools

    xq = nc.dram_tensor("xq", [L, D], BF16, kind="ExternalInput").ap()
    xk = nc.dram_tensor("xk", [L, D], BF16, kind="ExternalInput").ap()
    xv = nc.dram_tensor("xv", [L, D], BF16, kind="ExternalInput").ap()
    wq = nc.dram_tensor("wq", [D, CC], BF16, kind="ExternalInput").ap()
    wk = nc.dram_tensor("wk", [D, CC], BF16, kind="ExternalInput").ap()
    wv = nc.dram_tensor("wv", [D, CC], BF16, kind="ExternalInput").ap()
    wo = nc.dram_tensor("wo", [CC, D], BF16, kind="ExternalInput").ap()
    bq = nc.dram_tensor("bq", [CC], F32, kind="ExternalInput").ap()
    bk = nc.dram_tensor("bk", [CC], F32, kind="ExternalInput").ap()
    bv = nc.dram_tensor("bv", [CC], F32, kind="ExternalInput").ap()
    out = nc.dram_tensor("out", [D, L], BF16, kind="ExternalOutput").ap()

    EXP = mybir.ActivationFunctionType.Exp
    MULT = mybir.AluOpType.mult
    ADD = mybir.AluOpType.add

    wq_sb = sb1.tile([P, D // P, CC], BF16, tag="wq")
    wk_sb = sb1.tile([P, D // P, CC], BF16, tag="wk")
    wv_sb = sb1.tile([P, D // P, CC], BF16, tag="wv")
    wo_sb = sb1.tile([P, CC // P, D], BF16, tag="wo")

    bq_sb = sb1.tile([P, NP], F32, tag="bq")
    bk_sb = sb1.tile([P, NP], F32, tag="bk")
    nc.sync.dma_start(bq_sb[:], bq.rearrange("(o p) -> p o", p=P))
    nc.sync.dma_start(bk_sb[:], bk.rearrange("(o p) -> p o", p=P))
    bv_row = sb1.tile([1, CC], BF16, tag="bv")
    bv_f32 = sb1.tile([1, CC], F32, tag="bvf")
    nc.scalar.dma_start(bv_f32[:], bv[None, :])
    nc.vector.tensor_copy(bv_row[:], bv_f32[:])

    ones_row = sb1.tile([1, P], BF16, tag="ones_row")   # K=1 lhsT for V bias
    nc.vector.memset(ones_row[:], 1.0)
    ones64 = sb1.tile([P, DK], BF16, tag="ones64")      # rowsum+broadcast lhsT
    nc.vector.memset(ones64[:], 1.0)

    def load_x(x):
        """Transpose host-pre-cast bf16 x straight from DRAM, sync queue
        only: two DMA_TRANSPOSEs in flight on different hwdge queues corrupt
        each other (shared xbar state)."""
        xts = [xtp.tile([P, L], BF16, tag="xt", name=f"xt{i}")
               for i in range(D // P)]
        for c in range(D // P):
            nc.sync.dma_start_transpose(xts[c][:], x[:, c * P:(c + 1) * P])
        return xts

    qt_sb = sb1.tile([P, NP, L], BF16, tag="qt")   # [col-in-pair, pair, tok]
    kt_sb = sb1.tile([P, NP, L], BF16, tag="kt")   # [dk(A:0:64,B:64:128), pair, key]
    v_sb = sb1.tile([P, NKC, 8, DK], BF16, tag="v")  # [key, kc, head, dim]
    ct_sb = sb1.tile([P, NP, L], BF16, tag="ct")   # [col-in-pair, pair, tok]

    def load_w(w_sb, w, eng):
        eng.dma_start(w_sb[:], w.rearrange("(o p) c -> p o c", p=P))

    def proj_batch(w_sb, xts, dst, bias, p, tns):
        """One [128, len(tns)*512] psum group per (pair, tn-group): fewer
        big-pool allocations gating attention's score tiles, one batched
        bias-add."""
        n = len(tns)
        acc = big.tile([P, n, 512], F32, tag="sc", name="pacc")
        for j, tn in enumerate(tns):
            for c in range(D // P):
                nc.tensor.matmul(acc[:, j, :], w_sb[:, c, p * P:(p + 1) * P],
                                 xts[c][:, tn * 512:(tn + 1) * 512],
                                 start=(c == 0), stop=(c == D // P - 1),
                                 skip_group_check=(j > 0))
        t0 = tns[0]
        nc.vector.tensor_scalar_add(dst[:, p, t0 * 512:(t0 + n) * 512],
                                    acc[:], bias[:, p:p + 1])

    # K projection (p-major so pair 0 completes first)
    xts_k = load_x(xk)
    load_w(wk_sb, wk, nc.scalar)
    for p in range(NP):
        proj_batch(wk_sb, xts_k, kt_sb, bk_sb, p, (0, 1, 2))
        proj_batch(wk_sb, xts_k, kt_sb, bk_sb, p, (3,))

    # Q projection (q-chunk 0 first for all pairs, then the rest batched)
    xts_q = load_x(xq)
    load_w(wq_sb, wq, nc.scalar)
    for p in range(NP):
        proj_batch(wq_sb, xts_q, qt_sb, bq_sb, p, (0,))
    for p in range(NP):
        proj_batch(wq_sb, xts_q, qt_sb, bq_sb, p, (1, 2, 3))

    # V projection (tn-major; psum from the ctx pool so it doesn't contend
    # with attention's score tiles), straight into v_sb; attention consumes
    # v_sb[:, kc] chunks as they land.
    xts_v = load_x(xv)
    load_w(wv_sb, wv, nc.scalar)
    load_w(wo_sb, wo, nc.scalar)
    for tn in range(L // P):
        acc = ctxp.tile([P, 512], F32, tag="ctx", name="vacc")
        for c in range(D // P):
            nc.tensor.matmul(acc[:], xts_v[c][:, tn * P:(tn + 1) * P],
                             wv_sb[:, c, :], start=(c == 0), stop=False)
        nc.tensor.matmul(acc[:], ones_row[:], bv_row[:], start=False, stop=True)
        av = acc.rearrange("p (h m) -> p h m", m=DK)   # [128, 8, 64]
        nc.vector.tensor_copy(v_sb[:, tn, :, :], av)

    # ---- attention ----
    for qh in range(4):
        qs = slice(qh * 512, (qh + 1) * 512)
        for p in range(NP):
            ctx = ctxp.tile([P, 512], F32, tag="ctx")
            # S[:, h, :] accumulates sum_kc exp-tiles for head h in bf16.
            S = misc.tile([P, 2, 512], BF16, tag="S", bufs=4)
            s0 = 0
            while s0 < NSUB:
                ns = min(3, NSUB - s0)
                sc = big.tile([P, ns, 512], F32, tag="sc")
                for i in range(ns):
                    h, kc = (s0 + i) % 2, (s0 + i) // 2
                    hs = slice(DK * h, DK * (h + 1))
                    nc.tensor.matmul(sc[:, i, :],
                                     kt_sb[hs, p, kc * P:(kc + 1) * P],
                                     qt_sb[hs, p, qs], start=True, stop=True)
                pt = ptp.tile([P, ns, 512], BF16, tag="pt")
                nc.scalar.activation(pt[:], sc[:], EXP, scale=SCALE)
                for i in range(ns):
                    s = s0 + i
                    h, kc = s % 2, s // 2
                    nc.tensor.matmul(ctx[DK * h:DK * (h + 1), :],
                                     v_sb[:, kc, 2 * p + h, :], pt[:, i, :],
                                     start=(kc == 0),
                                     stop=(kc == NKC - 1),
                                     skip_group_check=(h == 1))
                # S accumulation: one paired (A,B) add + one single add per
                # tile; the pair starts at the even-s slot.
                j = 0 if s0 % 2 == 0 else 1
                if s0 == 0:
                    nc.vector.tensor_copy(S[:, 0:2, :], pt[:, 0:2, :])
                else:
                    nc.vector.tensor_tensor(S[:, 0:2, :], S[:, 0:2, :],
                                            pt[:, j:j + 2, :], ADD)
                if ns == 3:
                    k = 2 if j == 0 else 0
                    hk = (s0 + k) % 2
                    nc.vector.tensor_tensor(S[:, hk, :], S[:, hk, :],
                                            pt[:, k, :], ADD)
                s0 += ns
            # rowsum (partition reduce) + broadcast via all-ones matmul
            rb = ctxp.tile([P, 512], F32, tag="ctx")
            nc.tensor.matmul(rb[0:DK, :], ones64[:], S[:, 0, :],
                             start=True, stop=True)
            nc.tensor.matmul(rb[DK:P, :], ones64[:], S[:, 1, :],
                             start=True, stop=True, skip_group_check=True)
            rec = misc.tile([P, 512], F32, tag="rec", bufs=2)
            nc.vector.reciprocal_approx_fast(rec[:], rb[:])
            nc.vector.tensor_tensor(ct_sb[:, p, qs], ctx[:], rec[:], MULT)

    # ---- out projection: outT[oc, tok], 2-wide psum groups ----
    for g in range(16):
        oc, tn0 = g // 2, (g % 2) * 2
        acc = big.tile([P, 2, 512], F32, tag="sc")
        for j in range(2):
            tn = tn0 + j
            for p in range(NP):
                nc.tensor.matmul(acc[:, j, :],
                                 wo_sb[:, p, oc * P:(oc + 1) * P],
                                 ct_sb[:, p, tn * 512:(tn + 1) * 512],
                                 start=(p == 0), stop=(p == NP - 1))
        o_sb = osp.tile([P, 2, 512], BF16, tag="o")
        if g % 2 == 0:
            nc.vector.tensor_copy(o_sb[:], acc[:])
        else:
            nc.scalar.copy(o_sb[:], acc[:])
        nc.sync.dma_start(out[oc * P:(oc + 1) * P, tn0 * 512:(tn0 + 2) * 512],
                          o_sb[:])


def build_bass():
    nc = bacc.Bacc("TRN2", num_devices=8, debug=False)
    with tile.TileContext(nc) as tc:
        with (
            tc.tile_pool(name="sb1", bufs=1) as sb1,
            tc.tile_pool(name="xtp", bufs=16) as xtp,
            tc.tile_pool(name="ptp", bufs=6) as ptp,
            tc.tile_pool(name="osp", bufs=4) as osp,
            tc.tile_pool(name="misc", bufs=1) as misc,
            tc.tile_pool(name="dram", bufs=1, space="DRAM") as dram,
            tc.tile_pool(name="big", bufs=2, space="PSUM") as big,
            tc.tile_pool(name="ctxp", bufs=2, space="PSUM") as ctxp,
        ):
            build_attention_core(
                nc, tc, (sb1, xtp, ptp, osp, misc, dram, big, ctxp))
    nc.compile()
    return nc


_CACHE = {}


def _get_nc():
    if "nc" not in _CACHE:
        _CACHE["nc"] = build_bass()
    return _CACHE["nc"]


def make_in_maps(query, key, value, Wq, bq, Wk, bk, Wv, bv, Wo):
    f = np.ascontiguousarray
    in_maps = []
    for c in range(8):
        b, g = c // 2, c % 2
        cs = slice(g * CC, (g + 1) * CC)
        in_maps.append({
            "xq": f(query[b], dtype=BF16_NP),
            "xk": f(key[b], dtype=BF16_NP),
            "xv": f(value[b], dtype=BF16_NP),
            "wq": f(Wq[:, cs], dtype=BF16_NP),
            "wk": f(Wk[:, cs], dtype=BF16_NP),
            "wv": f(Wv[:, cs], dtype=BF16_NP),
            "wo": f(Wo[cs, :], dtype=BF16_NP),
            "bq": f(bq[cs], dtype=np.float32),
            "bk": f(bk[cs], dtype=np.float32),
            "bv": f(bv[cs], dtype=np.float32),
        })
    return in_maps


def kernel(query, key, value, Wq, bq, Wk, bk, Wv, bv, Wo, bo, **run_kwargs):
    query, key, value = np.asarray(query), np.asarray(key), np.asarray(value)
    Wq, Wk, Wv, Wo = np.asarray(Wq), np.asarray(Wk), np.asarray(Wv), np.asarray(Wo)
    bq, bk, bv, bo = np.asarray(bq), np.asarray(bk), np.asarray(bv), np.asarray(bo)
    nc = _get_nc()
    in_maps = make_in_maps(query, key, value, Wq, bq, Wk, bk, Wv, bv, Wo)
    res = run_bass_kernel_spmd(nc, in_maps, core_ids=list(range(8)), **run_kwargs)
    B = query.shape[0]
    out = np.empty((B, L, D), np.float32)
    for b in range(B):
        acc = (res.results[2 * b]["out"].astype(np.float32).T
               + res.results[2 * b + 1]["out"].astype(np.float32).T)
        out[b] = acc + bo[None, :].astype(np.float32)
    if run_kwargs:
        kernel.last_results = res
    return out
